# revision 1
# baseline (speedup 1.0000x reference)
"""MDN-RNN mixture-density loss kernel for Trainium2, SPMD over 8 NeuronCores.

Math (per token row i):
    means/logstds: [K, D] slices of s_mean/s_logstd rows
    z      = (target - mean_k) * exp(-logstd_k)
    logp_k = -0.5 * sum_d z^2 - sum_d logstd_k
    loss   = -mean_i logsumexp_k(log_mix_coeffs + logp_k)

Sharding: data-parallel on the token dim N=16384 -> 2048 rows per core,
no cross-device communication; each core emits nm = -max_k(score) and
S = sum_k exp(score+nm) per row packed [128, 2T]; the host finishes
loss = mean(nm - ln S) (a 16k-element ln, same category of work as the
partial-sum combine).

Engine split per 128-row tile (fp32, rows on partitions):
    DVE:    part of sls_k = sum_d logstd (grouped 3D reduce),
            diff = target(bcast over k) - mean, z = diff*e1 (3D mult),
            logsumexp smalls (scalar_tensor_tensor / reduce-max)
    ACT:    rest of sls_k (Copy w/ accumulate), e1 = exp(-logstd) in-place,
            per-k h_k = sum(z^2) via Square w/ accumulate, logsumexp exp

All scalar-engine functions (Copy/Exp/Square) live in one ACT table
set; there is no device Ln at all, so no activation-table swap ever —
the tail is just the two [128,16] result stores.

DMA note (hardware-measured): the DMA engines need full-row 21.7 KB
contiguous reads (split into ~10.9 KB packets) to reach ~405 GB/s
aggregate; per-k sub-row chunking drops the stream to 335-390 GB/s,
which is why tiles load as whole 3D transfers except the first/last
tile (chunked so the pipeline warms up / drains ~10 us faster).
"""

import sys

if "/opt/trn_rl_repo" not in sys.path:
    sys.path.insert(0, "/opt/trn_rl_repo")

import numpy as np

N = 16384
K = 5
D = 1088
KD = K * D
NCORES = 8
R = N // NCORES          # 2048 rows per core
P = 128                  # partitions
T = R // P               # 16 tiles per core

# number of per-k sum(logstd) reductions on ACT (rest grouped on DVE)
SLS_ACT_K = 1

_NC = None


def _build():
    import concourse.bacc as bacc
    import concourse.bass as bass
    import concourse.tile as tile
    from concourse import mybir

    AF = mybir.ActivationFunctionType
    AL = mybir.AluOpType
    AX = mybir.AxisListType
    f32 = mybir.dt.float32
    bf16 = mybir.dt.bfloat16

    nc = bacc.Bacc("TRN2", debug=False)
    tgt = nc.dram_tensor("tgt", [R, D], f32, kind="ExternalInput").ap()
    mean = nc.dram_tensor("mean", [R, KD], f32, kind="ExternalInput").ap()
    lstd = nc.dram_tensor("lstd", [R, KD], f32, kind="ExternalInput").ap()
    lmx = nc.dram_tensor("lmx", [P, T * K], f32, kind="ExternalInput").ap()
    out = nc.dram_tensor("res", [P, 2 * T], f32, kind="ExternalOutput").ap()

    with tile.TileContext(nc) as tc:
        with (
            tc.tile_pool(name="tgt_p", bufs=3) as tgt_p,
            tc.tile_pool(name="mean_p", bufs=3) as mean_p,
            tc.tile_pool(name="lstd_p", bufs=3) as lstd_p,
            tc.tile_pool(name="e1_p", bufs=2) as e1_p,
            tc.tile_pool(name="z_p", bufs=2) as z_p,
            tc.tile_pool(name="small_p", bufs=3) as small_p,
            tc.tile_pool(name="persist", bufs=1) as persist,
        ):
            t_lmx = persist.tile([P, T * K], f32)
            t_nmacc = persist.tile([P, T], f32)   # per-tile -max_k score
            t_sacc = persist.tile([P, T], f32)    # per-tile sum_k exp(score+nm)

            state = {}

            def emit_a(t):
                """Front stage: DMAs, sum(logstd), e1 = exp(-logstd), diff."""
                rows = slice(t * P, (t + 1) * P)
                split = t == T - 1
                t_tgt = tgt_p.tile([P, D], f32)
                t_mean = mean_p.tile([P, K, D], f32)
                t_lstd = lstd_p.tile([P, K, D], f32)
                mean3 = mean[rows].rearrange("p (k d) -> p k d", k=K)
                lstd3 = lstd[rows].rearrange("p (k d) -> p k d", k=K)
                if not split:
                    nc.sync.dma_start(out=t_lstd, in_=lstd3)
                    nc.sync.dma_start(out=t_tgt, in_=tgt[rows])
                    if t == 0:
                        # behind the first heavy loads; needed ~20us later
                        # by tile 0's logsumexp smalls
                        nc.sync.dma_start(out=t_lmx, in_=lmx)
                    nc.sync.dma_start(out=t_mean, in_=mean3)
                else:
                    # chunked so first compute starts after ~1/5 of the load
                    nc.sync.dma_start(out=t_lstd[:, 0, :], in_=lstd3[:, 0, :])
                    nc.sync.dma_start(out=t_tgt, in_=tgt[rows])
                    for k in range(1, K):
                        nc.sync.dma_start(out=t_lstd[:, k, :], in_=lstd3[:, k, :])
                    for k in range(K):
                        nc.sync.dma_start(out=t_mean[:, k, :], in_=mean3[:, k, :])

                t_sls = small_p.tile([P, K], f32)

                def emit_sls():
                    for k in range(SLS_ACT_K):
                        nc.scalar.activation(
                            out=t_lstd[:, k, :], in_=t_lstd[:, k, :], func=AF.Copy,
                            accum_out=t_sls[:, k : k + 1],
                        )
                    if SLS_ACT_K < K:
                        nc.vector.tensor_reduce(
                            out=t_sls[:, SLS_ACT_K:K], in_=t_lstd[:, SLS_ACT_K:K, :],
                            axis=AX.X, op=AL.add,
                        )

                if not split:
                    emit_sls()

                # e1 = exp(-logstd) into its own tile: no WAR against the
                # sls reads of lstd, so ACT can run it whenever lstd lands
                t_e1 = e1_p.tile([P, K, D], bf16)
                if not split:
                    nc.scalar.activation(out=t_e1, in_=t_lstd, func=AF.Exp, scale=-1.0)
                else:
                    # per-k: each exp fires as its lstd chunk lands (lstd
                    # chunks are issued before mean chunks, so e1_k is ready
                    # before mult_k needs it)
                    for k in range(K):
                        nc.scalar.activation(
                            out=t_e1[:, k, :], in_=t_lstd[:, k, :], func=AF.Exp,
                            scale=-1.0,
                        )

                # diff = target (broadcast over k) - mean, in place (DVE)
                tgt_b = bass.AP(
                    tensor=t_tgt.tensor, offset=t_tgt.offset,
                    ap=[t_tgt.ap[0], [0, K], t_tgt.ap[1]],
                )
                t_z = z_p.tile([P, K, D], bf16)
                if not split:
                    nc.vector.tensor_tensor(out=t_z, in0=tgt_b, in1=t_mean, op=AL.subtract)
                else:
                    # interleave sub/mult per k so each chain completes as
                    # its mean chunk lands instead of serializing at the end
                    for k in range(K):
                        nc.vector.tensor_tensor(
                            out=t_z[:, k, :], in0=t_tgt, in1=t_mean[:, k, :],
                            op=AL.subtract,
                        )
                        nc.vector.tensor_tensor(
                            out=t_z[:, k, :], in0=t_z[:, k, :],
                            in1=t_e1[:, k, :], op=AL.mult,
                        )
                    # sls after the sub/mult chains: it only feeds the final
                    # smalls, and ahead of them it head-of-line blocks the
                    # chunk-gated subs in the DVE FIFO
                    emit_sls()
                state[t] = (t_z, t_e1, t_sls)

            def emit_b(t):
                """Back stage: z, squares w/ accumulate, logsumexp smalls."""
                split = t == T - 1
                t_z, t_e1, t_sls = state.pop(t)
                t_h = small_p.tile([P, K], f32)
                if not split:
                    # z = diff * e1: all-bf16 3D mult hits the DVE 2x mode
                    nc.vector.tensor_tensor(out=t_z, in0=t_z, in1=t_e1, op=AL.mult)
                    # h_k = sum_d z^2 via ACT Square w/ accumulate (in place)
                    for k in range(K):
                        nc.scalar.activation(
                            out=t_z[:, k, :], in_=t_z[:, k, :], func=AF.Square,
                            accum_out=t_h[:, k : k + 1],
                        )
                else:
                    for k in range(K):
                        nc.scalar.activation(
                            out=t_z[:, k, :], in_=t_z[:, k, :], func=AF.Square,
                            accum_out=t_h[:, k : k + 1],
                        )

                # score_k = -0.5*h_k - sls_k + lmx_k ; nm = -max_k score
                t_q = small_p.tile([P, K], f32)
                nc.vector.scalar_tensor_tensor(
                    out=t_q, in0=t_h, scalar=-0.5, in1=t_sls,
                    op0=AL.mult, op1=AL.subtract,
                )
                t_c = small_p.tile([P, K], f32)
                nc.vector.tensor_tensor(
                    out=t_c, in0=t_q, in1=t_lmx[:, t * K : (t + 1) * K], op=AL.add
                )
                nc.vector.tensor_reduce(
                    out=t_nmacc[:, t : t + 1], in_=t_c, axis=AX.X, op=AL.max, negate=True
                )
                # S_t = sum_k exp(score + nm)
                t_e = small_p.tile([P, K], f32)
                nc.scalar.activation(
                    out=t_e, in_=t_c, func=AF.Exp, bias=t_nmacc[:, t : t + 1],
                    scale=1.0, accum_out=t_sacc[:, t : t + 1],
                )

            # software-pipelined emission: tile t+1's front stage is queued
            # before tile t's back stage so neither engine head-of-line
            # blocks on the cross-engine z/square seam
            emit_a(0)
            for t in range(T):
                if t + 1 < T - 1:
                    emit_a(t + 1)
                if t == T - 1:
                    emit_a(t)
                emit_b(t)

            # ship nm and S; the host finishes loss = mean(nm - ln S).
            # No device Ln -> single ACT table set, no tail table swap.
            nc.sync.dma_start(out=out[:, 0:T], in_=t_nmacc)
            nc.sync.dma_start(out=out[:, T : 2 * T], in_=t_sacc)

    nc.compile()
    return nc


def get_nc():
    global _NC
    if _NC is None:
        _NC = _build()
    return _NC


def make_in_maps(target, s_mean, s_logstd, log_mix_coeffs):
    target = np.ascontiguousarray(np.asarray(target, dtype=np.float32))
    s_mean = np.ascontiguousarray(np.asarray(s_mean, dtype=np.float32))
    s_logstd = np.ascontiguousarray(np.asarray(s_logstd, dtype=np.float32))
    lm = np.ascontiguousarray(np.asarray(log_mix_coeffs, dtype=np.float32))
    in_maps = []
    for c in range(NCORES):
        rows = slice(c * R, (c + 1) * R)
        # pack log-mix so tile t's [128, K] block sits at columns [t*K, (t+1)*K)
        lmx = lm[rows].reshape(T, P, K).transpose(1, 0, 2).reshape(P, T * K)
        in_maps.append({
            "tgt": np.ascontiguousarray(target[rows]),
            "mean": np.ascontiguousarray(s_mean[rows]),
            "lstd": np.ascontiguousarray(s_logstd[rows]),
            "lmx": np.ascontiguousarray(lmx),
        })
    return in_maps


def combine(results):
    # res[:, :T] = nm = -max_k score ; res[:, T:] = S = sum_k exp(score+nm)
    # lse = -nm + ln(S); loss = -mean(lse) = mean(nm - ln(S))
    total = 0.0
    for r in results:
        res = np.asarray(r["res"], dtype=np.float64)
        nm, s = res[:, :T], res[:, T:]
        total += float((nm - np.log(s)).sum())
    return np.float32(total / N)


def kernel(target, s_mean, s_logstd, log_mix_coeffs):
    from concourse.bass_utils import run_bass_kernel_spmd

    nc = get_nc()
    in_maps = make_in_maps(target, s_mean, s_logstd, log_mix_coeffs)
    res = run_bass_kernel_spmd(nc, in_maps, core_ids=list(range(NCORES)))
    return combine(res.results)



# revision 3
# speedup vs baseline: 1.3972x; 1.3972x over previous
"""MDN-RNN mixture-density loss kernel for Trainium2, SPMD over 8 NeuronCores.

Math (per token row i):
    means/logstds: [K, D] slices of s_mean/s_logstd rows
    z      = (target - mean_k) * exp(-logstd_k)
    logp_k = -0.5 * sum_d z^2 - sum_d logstd_k
    loss   = -mean_i logsumexp_k(log_mix_coeffs + logp_k)

Sharding: data-parallel on the token dim N=16384 -> 2048 rows per core,
no cross-device communication; each core emits nm = -max_k(score) and
S = sum_k exp(score+nm) per row packed [128, 2T]; the host finishes
loss = mean(nm - ln S).

The host uploads bf16 inputs packed per row as [lstd | tgt | mean]
(one contiguous 23.9KB stream per row): HBM traffic halves vs f32 and
each 128-row tile is a single contiguous DMA. bf16 rounding of the
inputs perturbs each row's logsumexp by a zero-mean ~1e-3 relative
amount that averages out over 16384 rows (measured ~1e-4 on the loss).

Engine split per 128-row tile (all big ops bf16 -> DVE 2x mode):
    ACT:    e1 = exp(-lstd), squares w/ accumulate for k=0..2,
            final exp(score+nm) w/ accumulate
    DVE:    diff = target(bcast over k) - mean, z = diff*e1,
            fused square+sum (tensor_tensor_reduce) for k=3..4,
            sls = grouped sum_d logstd, logsumexp smalls
"""

import sys

if "/opt/trn_rl_repo" not in sys.path:
    sys.path.insert(0, "/opt/trn_rl_repo")

import numpy as np

N = 16384
K = 5
D = 1088
KD = K * D
NCORES = 8
R = N // NCORES          # 2048 rows per core
P = 128                  # partitions
T = R // P               # 16 tiles per core

PK = KD + D + KD         # 11968 packed row: [lstd | tgt | mean]
TG0, TG1 = KD, KD + D
MN0 = KD + D

ACT_K = 5                # squares on ACT (k < ACT_K); rest fused on DVE

_NC = None


def _build():
    import concourse.bacc as bacc
    import concourse.bass as bass
    import concourse.tile as tile
    from concourse import mybir

    AF = mybir.ActivationFunctionType
    AL = mybir.AluOpType
    AX = mybir.AxisListType
    f32 = mybir.dt.float32
    bf16 = mybir.dt.bfloat16

    nc = bacc.Bacc("TRN2", debug=False)
    pk = nc.dram_tensor("pk", [R, PK], bf16, kind="ExternalInput").ap()
    lmx = nc.dram_tensor("lmx", [P, T * K], f32, kind="ExternalInput").ap()
    out = nc.dram_tensor("res", [P, 2 * T], f32, kind="ExternalOutput").ap()

    with tile.TileContext(nc) as tc:
        with (
            tc.tile_pool(name="all_p", bufs=4) as all_p,
            tc.tile_pool(name="e1_p", bufs=2) as e1_p,
            tc.tile_pool(name="z_p", bufs=2) as z_p,
            tc.tile_pool(name="small_p", bufs=3) as small_p,
            tc.tile_pool(name="persist", bufs=1) as persist,
        ):
            t_lmx = persist.tile([P, T * K], f32)
            t_nmacc = persist.tile([P, T], f32)   # per-tile -max_k score
            t_sacc = persist.tile([P, T], f32)    # per-tile sum_k exp(score+nm)

            state = {}

            def emit_a(t):
                """Front stage: DMA, e1 = exp(-lstd), diff = tgt - mean."""
                rows = slice(t * P, (t + 1) * P)
                t_all = all_p.tile([P, PK], bf16)
                if t == 0:
                    # lstd chunk first so exp starts ~4us earlier
                    nc.sync.dma_start(out=t_all[:, 0:KD], in_=pk[rows, 0:KD])
                    nc.sync.dma_start(out=t_lmx, in_=lmx)
                    nc.sync.dma_start(out=t_all[:, KD:PK], in_=pk[rows, KD:PK])
                else:
                    nc.sync.dma_start(out=t_all, in_=pk[rows])

                t_e1 = e1_p.tile([P, KD], bf16)
                nc.scalar.activation(
                    out=t_e1, in_=t_all[:, 0:KD], func=AF.Exp, scale=-1.0
                )

                # diff = target (broadcast over k) - mean (3D views, all bf16)
                t_tg = t_all[:, TG0:TG1]
                tgt_b = bass.AP(
                    tensor=t_tg.tensor, offset=t_tg.offset,
                    ap=[t_tg.ap[0], [0, K], t_tg.ap[1]],
                )
                mean3 = t_all[:, MN0:PK].rearrange("p (k d) -> p k d", k=K)
                t_z = z_p.tile([P, K, D], bf16)
                nc.vector.tensor_tensor(out=t_z, in0=tgt_b, in1=mean3, op=AL.subtract)
                state[t] = (t_all, t_e1, t_z)

            def emit_b(t):
                """Back stage: z, per-k sum z^2, sls, logsumexp smalls."""
                t_all, t_e1, t_z = state.pop(t)
                z2 = t_z.rearrange("p k d -> p (k d)")
                # z = diff * e1 (bf16 2x mode)
                nc.vector.tensor_tensor(out=z2, in0=z2, in1=t_e1, op=AL.mult)

                t_h = small_p.tile([P, K], f32)
                # DVE fused square+sum for the last K-ACT_K components
                for k in range(ACT_K, K):
                    nc.vector.tensor_tensor_reduce(
                        out=t_z[:, k, :], in0=t_z[:, k, :], in1=t_z[:, k, :],
                        scale=1.0, scalar=0.0, op0=AL.mult, op1=AL.add,
                        accum_out=t_h[:, k : k + 1],
                    )
                # ACT squares w/ accumulate for the first ACT_K components
                for k in range(ACT_K):
                    nc.scalar.activation(
                        out=t_z[:, k, :], in_=t_z[:, k, :], func=AF.Square,
                        accum_out=t_h[:, k : k + 1],
                    )
                # sls_k = sum_d logstd (grouped 3D reduce, f32 out)
                t_sls = small_p.tile([P, K], f32)
                lstd3 = t_all[:, 0:KD].rearrange("p (k d) -> p k d", k=K)
                nc.vector.tensor_reduce(
                    out=t_sls, in_=lstd3, axis=AX.X, op=AL.add
                )

                # score_k = -0.5*h_k - sls_k + lmx_k ; nm = -max_k score
                t_q = small_p.tile([P, K], f32)
                nc.vector.scalar_tensor_tensor(
                    out=t_q, in0=t_h, scalar=-0.5, in1=t_sls,
                    op0=AL.mult, op1=AL.subtract,
                )
                t_c = small_p.tile([P, K], f32)
                nc.vector.tensor_tensor(
                    out=t_c, in0=t_q, in1=t_lmx[:, t * K : (t + 1) * K], op=AL.add
                )
                nc.vector.tensor_reduce(
                    out=t_nmacc[:, t : t + 1], in_=t_c, axis=AX.X, op=AL.max, negate=True
                )
                # S_t = sum_k exp(score + nm)
                t_e = small_p.tile([P, K], f32)
                nc.scalar.activation(
                    out=t_e, in_=t_c, func=AF.Exp, bias=t_nmacc[:, t : t + 1],
                    scale=1.0, accum_out=t_sacc[:, t : t + 1],
                )

            # software-pipelined emission: tile t+1's front stage is queued
            # before tile t's back stage so ACT's exp(t+1) overlaps DVE's
            # mult/reduce chain of tile t
            emit_a(0)
            for t in range(T):
                if t + 1 < T:
                    emit_a(t + 1)
                emit_b(t)

            # ship nm and S; the host finishes loss = mean(nm - ln S)
            nc.sync.dma_start(out=out[:, 0:T], in_=t_nmacc)
            nc.sync.dma_start(out=out[:, T : 2 * T], in_=t_sacc)

    nc.compile()
    return nc


def get_nc():
    global _NC
    if _NC is None:
        _NC = _build()
    return _NC


def make_in_maps(target, s_mean, s_logstd, log_mix_coeffs):
    import ml_dtypes

    BF = ml_dtypes.bfloat16
    tb = np.asarray(target, dtype=np.float32).astype(BF)
    mb = np.asarray(s_mean, dtype=np.float32).astype(BF)
    lb = np.asarray(s_logstd, dtype=np.float32).astype(BF)
    lm = np.ascontiguousarray(np.asarray(log_mix_coeffs, dtype=np.float32))
    in_maps = []
    for c in range(NCORES):
        rows = slice(c * R, (c + 1) * R)
        pk = np.empty((R, PK), dtype=BF)
        pk[:, 0:KD] = lb[rows]
        pk[:, TG0:TG1] = tb[rows]
        pk[:, MN0:PK] = mb[rows]
        # pack log-mix so tile t's [128, K] block sits at columns [t*K, (t+1)*K)
        lmx = lm[rows].reshape(T, P, K).transpose(1, 0, 2).reshape(P, T * K)
        in_maps.append({
            "pk": pk,
            "lmx": np.ascontiguousarray(lmx),
        })
    return in_maps


def combine(results):
    # res[:, :T] = nm = -max_k score ; res[:, T:] = S = sum_k exp(score+nm)
    # lse = -nm + ln(S); loss = -mean(lse) = mean(nm - ln(S))
    total = 0.0
    for r in results:
        res = np.asarray(r["res"], dtype=np.float64)
        nm, s = res[:, :T], res[:, T:]
        total += float((nm - np.log(s)).sum())
    return np.float32(total / N)


def kernel(target, s_mean, s_logstd, log_mix_coeffs):
    from concourse.bass_utils import run_bass_kernel_spmd

    nc = get_nc()
    in_maps = make_in_maps(target, s_mean, s_logstd, log_mix_coeffs)
    res = run_bass_kernel_spmd(nc, in_maps, core_ids=list(range(NCORES)))
    return combine(res.results)
